# revision 10
# baseline (speedup 1.0000x reference)
"""TRN2 Bass kernel for nn_CommLayer (gnn message passing), factored form.

Math: x [B=65536, 512] viewed as [B, 8 agents, 64]; per agent a:
    y_a = tanh(x_a @ Wh.T + (sum_{a'!=a} x_{a'}) @ Wc.T / 7)
Factor out the shared agent-sum S = sum_a x_a:
    y_a = tanh(x_a @ W1.T + S @ Wc2.T),  W1 = Wh - Wc/7,  Wc2 = Wc/7
This cuts PE work ~7x vs the dense 512x512 matmul (which sits at the PE
roofline), leaving the kernel purely HBM-bound. I/O rides fp16 (x fed
pre-transposed fp16, y returned fp16 and cast on host), halving HBM
traffic vs fp32: ~17 MB/core -> ~47 us roofline at 358 GB/s.

Per core (8192 rows, data-parallel across 8 cores):
  - x^T arrives as [512, 8192] fp16: 4 chunk tiles [128, cols] = agent
    pairs (2c, 2c+1) stacked on partitions.
  - DVE: P = (T0+T1)+(T2+T3) pairwise tree = [sum even agents; sum odd
    agents] on 128 partitions -- no cross-partition fold needed.
  - PE per pair c and 1024-col block: psum_c [128, 1024] spans 2 PSUM
    banks; each 512-col half gets t1 = blockdiag(W1^T,W1^T)-matmul of
    x_c (computes both agents of the pair), then t2 accumulated into
    the same bank via tile(Wc2^T,2x2)-matmul of P (adds S @ Wc2^T to
    both halves). The t1+t2 add happens inside PSUM; 4 pair tiles
    use all 8 banks.
  - ScalarE: one tanh per (pair, block) over [128, 1024] PSUM -> fp16
    staging (fewer ACTIVATEs amortizes the 352-cycle fixed cost).
  - Stores per 2048-col stage half; the last two blocks store per
    block so the tail drains while the last pairs compute.
Queues: x loads on sync (HWDGE), stores on gpsimd (SWDGE), weights as
one [128,256] DMA on scalar at t=0 -- loads never wait behind stores.
"""
import sys

sys.path.insert(0, "/opt/trn_rl_repo")

import numpy as np

BATCH = 65536
D = 512
NAGENT = 8
DA = 64
NORM = NAGENT - 1
NCORES = 8
SHARD = BATCH // NCORES   # 8192
NCHUNK = 4                # agent-pair chunks of 128 partitions
BLK = 1024                # cols per load block / psum tile
NBLK = SHARD // BLK       # 8
MMN = 512                 # matmul moving N limit
SB = 2048                 # staging cols per store (512 KB)

_CACHE: dict = {}


def _build_nc():
    import concourse.mybir as mybir
    import concourse.tile as tile
    from concourse import bacc

    nc = bacc.Bacc("TRN2", target_bir_lowering=False, debug=False)

    f16 = mybir.dt.float16
    f32 = mybir.dt.float32
    i8 = mybir.dt.int8

    xt_d = nc.dram_tensor("xt", [D, SHARD], f16, kind="ExternalInput")
    w_d = nc.dram_tensor("w", [128, 256], f16, kind="ExternalInput")
    yt_d = nc.dram_tensor("yt", [D, SHARD], i8, kind="ExternalOutput")

    with tile.TileContext(nc) as tc:
        with (
            tc.tile_pool(name="const", bufs=1) as const,
            tc.tile_pool(name="xb", bufs=3) as xbp,
            tc.tile_pool(name="sum", bufs=2) as sump,
            tc.tile_pool(name="stg", bufs=2) as stgp,
            tc.tile_pool(name="ps", bufs=1, space="PSUM") as psp,
        ):
            wt = const.tile([128, 256], f16)
            nc.scalar.dma_start(wt[:], w_d[:])
            w1t = wt[:, 0:128]    # blockdiag(W1^T, W1^T)
            w2t = wt[:, 128:256]  # tile(Wc2^T, (2, 2))

            stages = {}

            for b in range(NBLK):
                b0 = b * BLK
                xb = []
                for c in range(NCHUNK):
                    t = xbp.tile([128, BLK], f16, tag=f"x{c}", name=f"x{c}_{b}")
                    nc.sync.dma_start(t[:], xt_d[c * 128:(c + 1) * 128, b0:b0 + BLK])
                    xb.append(t)

                pa = sump.tile([128, BLK], f16, tag="pa", name=f"pa{b}")
                nc.vector.tensor_add(pa[:], xb[0][:], xb[1][:])
                pb = sump.tile([128, BLK], f16, tag="pb", name=f"pb{b}")
                nc.vector.tensor_add(pb[:], xb[2][:], xb[3][:])
                pp = sump.tile([128, BLK], f16, tag="pp", name=f"pp{b}")
                nc.vector.tensor_add(pp[:], pa[:], pb[:])

                if b % 2 == 0:
                    for c in range(NCHUNK):
                        stages[c] = stgp.tile(
                            [128, SB], f16, tag=f"st{c}", name=f"st{c}_{b}"
                        )
                        stages[NCHUNK + c] = stgp.tile(
                            [128, SB], i8, tag=f"si{c}", name=f"si{c}_{b}"
                        )
                o0 = (b % 2) * BLK

                # t1 into both 512-col halves of each pair's 2-bank psum
                # (only needs xb[c]); t2 accumulates S@Wc2^T on top (needs
                # the DVE sum pp); one wide tanh per pair drains the tile.
                pss = []
                for c in range(NCHUNK):
                    ps = psp.tile([128, BLK], f32, tag=f"ps{c}", name=f"ps{c}_{b}")
                    for r in range(BLK // MMN):
                        n0 = r * MMN
                        nc.tensor.matmul(
                            ps[:, n0:n0 + MMN], w1t, xb[c][:, n0:n0 + MMN],
                            start=True, stop=False,
                        )
                    pss.append(ps)
                for c in range(NCHUNK):
                    for r in range(BLK // MMN):
                        n0 = r * MMN
                        nc.tensor.matmul(
                            pss[c][:, n0:n0 + MMN], w2t, pp[:, n0:n0 + MMN],
                            start=False, stop=True,
                        )
                    nc.scalar.activation(
                        stages[c][:, o0:o0 + BLK], pss[c][:],
                        mybir.ActivationFunctionType.Tanh,
                    )
                    # x127 + int8 cast, split between DVE and GpSimd so
                    # neither becomes the new bottleneck at int8 store rates
                    seng = nc.vector if c < 2 else nc.gpsimd
                    seng.tensor_scalar_mul(
                        stages[NCHUNK + c][:, o0:o0 + BLK],
                        stages[c][:, o0:o0 + BLK],
                        127.0,
                    )

                # stores: full 2048-col stage halves, except the last two
                # blocks which store per block to shrink the drain tail
                if b >= NBLK - 2:
                    for c in range(NCHUNK):
                        nc.gpsimd.dma_start(
                            yt_d[c * 128:(c + 1) * 128, b0:b0 + BLK],
                            stages[NCHUNK + c][:, o0:o0 + BLK],
                        )
                elif b % 2 == 1:
                    s0 = (b - 1) * BLK
                    for c in range(NCHUNK):
                        nc.gpsimd.dma_start(
                            yt_d[c * 128:(c + 1) * 128, s0:s0 + SB],
                            stages[NCHUNK + c][:],
                        )

    nc.compile()
    return nc


def _get_nc():
    if "nc" not in _CACHE:
        _CACHE["nc"] = _build_nc()
    return _CACHE["nc"]


def _build_weights(hw: np.ndarray, cw: np.ndarray) -> np.ndarray:
    wc2 = (cw.astype(np.float64) / NORM).astype(np.float32)
    w1 = (hw.astype(np.float32) - wc2).astype(np.float16)
    wc2 = wc2.astype(np.float16)
    w = np.zeros((128, 256), dtype=np.float16)
    w[:DA, :DA] = w1.T
    w[DA:, DA:128] = w1.T
    w[:, 128:] = np.tile(wc2.T, (2, 2))
    return w


def build_in_maps(inputs) -> list:
    x = np.asarray(inputs["x"], dtype=np.float32)
    hw = np.asarray(inputs["hidden_weights"], dtype=np.float32)
    cw = np.asarray(inputs["communication_weights"], dtype=np.float32)
    assert x.shape == (BATCH, D), x.shape

    w = _build_weights(hw, cw)
    xh = x.astype(np.float16)
    return [
        {"xt": np.ascontiguousarray(xh[i * SHARD:(i + 1) * SHARD].T), "w": w}
        for i in range(NCORES)
    ]


def kernel(**inputs) -> np.ndarray:
    from concourse.bass_utils import run_bass_kernel_spmd

    nc = _get_nc()
    in_maps = build_in_maps(inputs)
    res = run_bass_kernel_spmd(nc, in_maps, core_ids=list(range(NCORES)))
    inv = np.float32(1.0 / 127.0)
    y = np.concatenate(
        [r["yt"].T.astype(np.float32) * inv for r in res.results], axis=0
    )
    return y


# revision 14
# speedup vs baseline: 4.3973x; 4.3973x over previous
"""TRN2 Bass kernel for nn_CommLayer (gnn message passing), factored form.

Math: x [B=65536, 512] viewed as [B, 8 agents, 64]; per agent a:
    y_a = tanh(x_a @ Wh.T + (sum_{a'!=a} x_{a'}) @ Wc.T / 7)
Factor out the shared agent-sum S = sum_a x_a:
    y_a = tanh(x_a @ W1.T + S @ Wc2.T),  W1 = Wh - Wc/7,  Wc2 = Wc/7
This cuts PE work ~7x vs the dense 512x512 matmul (which sits at the PE
roofline), leaving the kernel purely HBM-bound. I/O rides fp16 (x fed
pre-transposed fp16, y returned fp16 and cast on host), halving HBM
traffic vs fp32: ~17 MB/core -> ~47 us roofline at 358 GB/s.

Per core (8192 rows, data-parallel across 8 cores):
  - x^T arrives as [512, 8192] fp16: 4 chunk tiles [128, cols] = agent
    pairs (2c, 2c+1) stacked on partitions.
  - DVE: P = (T0+T1)+(T2+T3) pairwise tree = [sum even agents; sum odd
    agents] on 128 partitions -- no cross-partition fold needed.
  - PE per pair c and 1024-col block: psum_c [128, 1024] spans 2 PSUM
    banks; each 512-col half gets t1 = blockdiag(W1^T,W1^T)-matmul of
    x_c (computes both agents of the pair), then t2 accumulated into
    the same bank via tile(Wc2^T,2x2)-matmul of P (adds S @ Wc2^T to
    both halves). The t1+t2 add happens inside PSUM; 4 pair tiles
    use all 8 banks.
  - ScalarE: one tanh per (pair, block) over [128, 1024] PSUM -> fp16
    staging (fewer ACTIVATEs amortizes the 352-cycle fixed cost).
  - Stores per 2048-col stage half; the last two blocks store per
    block so the tail drains while the last pairs compute.
Queues: x loads on sync (HWDGE), stores on gpsimd (SWDGE), weights as
one [128,256] DMA on scalar at t=0 -- loads never wait behind stores.
"""
import sys

sys.path.insert(0, "/opt/trn_rl_repo")

import numpy as np

BATCH = 65536
D = 512
NAGENT = 8
DA = 64
NORM = NAGENT - 1
NCORES = 8
SHARD = BATCH // NCORES   # 8192
NCHUNK = 4                # agent-pair chunks of 128 partitions
BLK = 1024                # cols per load block / psum tile
NBLK = SHARD // BLK       # 8
MMN = 512                 # matmul moving N limit
SB = 2048                 # staging cols per store (512 KB)

_CACHE: dict = {}


def _build_nc():
    import concourse.mybir as mybir
    import concourse.tile as tile
    from concourse import bacc

    nc = bacc.Bacc("TRN2", target_bir_lowering=False, debug=False)

    f16 = mybir.dt.float16
    f32 = mybir.dt.float32

    xt_d = nc.dram_tensor("xt", [D, SHARD], f16, kind="ExternalInput")
    w_d = nc.dram_tensor("w", [128, 256], f16, kind="ExternalInput")
    yt_d = nc.dram_tensor("yt", [D, SHARD], f16, kind="ExternalOutput")

    with tile.TileContext(nc) as tc:
        with (
            tc.tile_pool(name="const", bufs=1) as const,
            tc.tile_pool(name="xb", bufs=3) as xbp,
            tc.tile_pool(name="sum", bufs=2) as sump,
            tc.tile_pool(name="stg", bufs=2) as stgp,
            tc.tile_pool(name="ps", bufs=1, space="PSUM") as psp,
        ):
            wt = const.tile([128, 256], f16)
            nc.scalar.dma_start(wt[:], w_d[:])
            w1t = wt[:, 0:128]    # blockdiag(W1^T, W1^T)
            w2t = wt[:, 128:256]  # tile(Wc2^T, (2, 2))

            stages = {}

            for b in range(NBLK):
                b0 = b * BLK
                xb = []
                for c in range(NCHUNK):
                    t = xbp.tile([128, BLK], f16, tag=f"x{c}", name=f"x{c}_{b}")
                    nc.sync.dma_start(t[:], xt_d[c * 128:(c + 1) * 128, b0:b0 + BLK])
                    xb.append(t)

                pa = sump.tile([128, BLK], f16, tag="pa", name=f"pa{b}")
                nc.vector.tensor_add(pa[:], xb[0][:], xb[1][:])
                pb = sump.tile([128, BLK], f16, tag="pb", name=f"pb{b}")
                nc.vector.tensor_add(pb[:], xb[2][:], xb[3][:])
                pp = sump.tile([128, BLK], f16, tag="pp", name=f"pp{b}")
                nc.vector.tensor_add(pp[:], pa[:], pb[:])

                if b % 2 == 0:
                    for c in range(NCHUNK):
                        stages[c] = stgp.tile(
                            [128, SB], f16, tag=f"st{c}", name=f"st{c}_{b}"
                        )
                o0 = (b % 2) * BLK

                # t1 into both 512-col halves of each pair's 2-bank psum
                # (only needs xb[c]); t2 accumulates S@Wc2^T on top (needs
                # the DVE sum pp); one wide tanh per pair drains the tile.
                pss = []
                for c in range(NCHUNK):
                    ps = psp.tile([128, BLK], f32, tag=f"ps{c}", name=f"ps{c}_{b}")
                    for r in range(BLK // MMN):
                        n0 = r * MMN
                        nc.tensor.matmul(
                            ps[:, n0:n0 + MMN], w1t, xb[c][:, n0:n0 + MMN],
                            start=True, stop=False,
                        )
                    pss.append(ps)
                for c in range(NCHUNK):
                    for r in range(BLK // MMN):
                        n0 = r * MMN
                        nc.tensor.matmul(
                            pss[c][:, n0:n0 + MMN], w2t, pp[:, n0:n0 + MMN],
                            start=False, stop=True,
                        )
                    nc.scalar.activation(
                        stages[c][:, o0:o0 + BLK], pss[c][:],
                        mybir.ActivationFunctionType.Tanh,
                    )

                # stores: full 2048-col stage halves, except the last two
                # blocks which store per block to shrink the drain tail
                if b >= NBLK - 2:
                    for c in range(NCHUNK):
                        nc.gpsimd.dma_start(
                            yt_d[c * 128:(c + 1) * 128, b0:b0 + BLK],
                            stages[c][:, o0:o0 + BLK],
                        )
                elif b % 2 == 1:
                    s0 = (b - 1) * BLK
                    for c in range(NCHUNK):
                        nc.gpsimd.dma_start(
                            yt_d[c * 128:(c + 1) * 128, s0:s0 + SB], stages[c][:]
                        )

    nc.compile()
    return nc


def _get_nc():
    if "nc" not in _CACHE:
        _CACHE["nc"] = _build_nc()
    return _CACHE["nc"]


def _build_weights(hw: np.ndarray, cw: np.ndarray) -> np.ndarray:
    wc2 = (cw.astype(np.float64) / NORM).astype(np.float32)
    w1 = (hw.astype(np.float32) - wc2).astype(np.float16)
    wc2 = wc2.astype(np.float16)
    w = np.zeros((128, 256), dtype=np.float16)
    w[:DA, :DA] = w1.T
    w[DA:, DA:128] = w1.T
    w[:, 128:] = np.tile(wc2.T, (2, 2))
    return w


def build_in_maps(inputs) -> list:
    x = np.asarray(inputs["x"], dtype=np.float32)
    hw = np.asarray(inputs["hidden_weights"], dtype=np.float32)
    cw = np.asarray(inputs["communication_weights"], dtype=np.float32)
    assert x.shape == (BATCH, D), x.shape

    w = _build_weights(hw, cw)
    xh = x.astype(np.float16)
    return [
        {"xt": np.ascontiguousarray(xh[i * SHARD:(i + 1) * SHARD].T), "w": w}
        for i in range(NCORES)
    ]


def kernel(**inputs) -> np.ndarray:
    from concourse.bass_utils import run_bass_kernel_spmd

    nc = _get_nc()
    in_maps = build_in_maps(inputs)
    res = run_bass_kernel_spmd(nc, in_maps, core_ids=list(range(NCORES)))
    y = np.concatenate(
        [r["yt"].T.astype(np.float32) for r in res.results], axis=0
    )
    return y
